# revision 7
# baseline (speedup 1.0000x reference)
"""Multi-head attention kernel for Trainium2, 8 NeuronCores.

Problem: B=4, T=2048, D=1024, H=16 heads, head_dim=64.
Sharding: core c -> batch b = c//2, head group g = c%2 (8 heads each).
Each core computes QKV projections for its 512 features and full
attention for its 8 heads over its batch. No cross-core communication.

Per-core layout (all matmul inputs bf16, fp32 accumulation):
  - x is passed transposed+chunked: xt[p, dc, t] = x[b, t, 128*dc+p]
  - weights passed chunked:  wq[p, dc, f] = Wq[128*dc+p, 512*g+f]
  - Q^T/K^T computed feature-major [feat, t] so attention scores
    S^T[k, q] = sum_d K^T[d, k] Q^T[d, q] come out with k on partitions
  - V computed in natural [t, f] layout, augmented with a ones column:
    PV matmul accumulates [65, 512] where row 64 = softmax denominator
  - softmax needs no max subtraction: |S/8| <= ~7 for N(0,1) inputs
  - output written per head as O^T [64, t]; host transposes/concats
"""

import os
import sys

for _p in ("/opt/trn_rl_repo", "/opt/pypackages"):
    if _p not in sys.path:
        sys.path.insert(0, _p)

import numpy as np
import ml_dtypes

B, T, D, H = 4, 2048, 1024, 16
HD = D // H            # 64 head dim
N_CORES = 8
G = 2                  # head groups (cores per batch)
F = D // G             # 512 features per core
HPC = H // G           # 8 heads per core
P = 128
DC = D // P            # 8 contraction chunks
NPAIR = HPC // 2       # 4 head pairs per core
QC = 512               # query-chunk (columns per score matmul)
NQC = T // QC          # 4 query chunks
NKT = T // P           # 16 key tiles

BF16 = ml_dtypes.bfloat16

_compiled = None  # (nc,) cached across calls in one process


def _build():
    import concourse.bass as bass
    import concourse.tile as tile
    from concourse import bacc, mybir

    fp32 = mybir.dt.float32
    bf16 = mybir.dt.bfloat16
    Exp = mybir.ActivationFunctionType.Exp

    nc = bacc.Bacc("TRN2", target_bir_lowering=False, debug=False,
                   num_devices=N_CORES)

    xt = nc.dram_tensor("xt", [P, DC, T], bf16, kind="ExternalInput").ap()
    wq = nc.dram_tensor("wq", [P, DC, F], bf16, kind="ExternalInput").ap()
    wk = nc.dram_tensor("wk", [P, DC, F], bf16, kind="ExternalInput").ap()
    wv = nc.dram_tensor("wv", [P, DC, F], bf16, kind="ExternalInput").ap()
    bq = nc.dram_tensor("bq", [P, NPAIR], fp32, kind="ExternalInput").ap()
    bk = nc.dram_tensor("bk", [P, NPAIR], fp32, kind="ExternalInput").ap()
    bv = nc.dram_tensor("bv", [P, F], fp32, kind="ExternalInput").ap()
    o = nc.dram_tensor("o", [HPC, HD, T], fp32, kind="ExternalOutput").ap()

    with tile.TileContext(nc) as tc:
        with (
            tc.tile_pool(name="singles", bufs=1) as singles,
            tc.tile_pool(name="es", bufs=4) as es_pool,
            tc.tile_pool(name="stage", bufs=2) as stage_pool,
            tc.tile_pool(name="norm", bufs=2) as norm_pool,
            tc.tile_pool(name="sps", bufs=2, space="PSUM") as sps_pool,
            tc.tile_pool(name="pv", bufs=1, space="PSUM") as pv_pool,
            tc.tile_pool(name="qkv", bufs=2, space="PSUM") as qkv_pool,
        ):
            # ---- persistent SBUF tensors ----
            xt_sb = singles.tile([P, DC, T], bf16, tag="xt")
            wq_sb = singles.tile([P, DC, F], bf16, tag="wq")
            wk_sb = singles.tile([P, DC, F], bf16, tag="wk")
            wv_sb = singles.tile([P, DC, F], bf16, tag="wv")
            bq_sb = singles.tile([P, NPAIR], fp32, tag="bq")
            bk_sb = singles.tile([P, NPAIR], fp32, tag="bk")
            bv_sb = singles.tile([P, F], fp32, tag="bv")
            # per-pair Q^T/K^T [feat-in-pair, t] and V [t-in-ktile, kt, hp, 65]
            qt_sb = [singles.tile([P, T], bf16, tag=f"qt{j}", name=f"qt{j}")
                     for j in range(NPAIR)]
            kt_sb = [singles.tile([P, T], bf16, tag=f"kt{j}", name=f"kt{j}")
                     for j in range(NPAIR)]
            v_sb = [singles.tile([P, NKT, 2, HD + 1], bf16, tag=f"v{j}",
                                 name=f"v{j}")
                    for j in range(NPAIR)]
            # normalize staging, separate per head-slot (a/b):
            # rzs holds 1/Z on partition 64, rz0 the same row moved to
            # partition 0 (sb->sb DMA), rzb the broadcast across 0..63
            rzs = [singles.tile([HD + 1, QC], fp32, tag=f"rzs{i}",
                                name=f"rzs{i}") for i in range(2)]
            rz0 = [singles.tile([1, QC], fp32, tag=f"rz0{i}",
                                name=f"rz0{i}") for i in range(2)]
            rzb = [singles.tile([HD, QC], fp32, tag=f"rzb{i}",
                                name=f"rzb{i}") for i in range(2)]

            nc.sync.dma_start(out=xt_sb[:], in_=xt[:])
            nc.sync.dma_start(out=wq_sb[:], in_=wq[:])
            nc.sync.dma_start(out=wk_sb[:], in_=wk[:])
            nc.sync.dma_start(out=wv_sb[:], in_=wv[:])
            nc.sync.dma_start(out=bq_sb[:], in_=bq[:])
            nc.sync.dma_start(out=bk_sb[:], in_=bk[:])
            nc.sync.dma_start(out=bv_sb[:], in_=bv[:])
            for j in range(NPAIR):
                nc.vector.memset(v_sb[j][:, :, :, HD:HD + 1], 1.0)

            def emit_qk_proj(j):
                """Q^T/K^T rows for pair j: psum [f=128, t=512] per t-chunk."""
                for w_sb, b_sb, dst in ((wq_sb, bq_sb, qt_sb[j]),
                                        (wk_sb, bk_sb, kt_sb[j])):
                    for tcn in range(T // 512):
                        ps = qkv_pool.tile([P, 512], fp32, tag="qkv")
                        for dc in range(DC):
                            nc.tensor.matmul(
                                ps[:],
                                w_sb[:, dc, P * j:P * (j + 1)],
                                xt_sb[:, dc, 512 * tcn:512 * (tcn + 1)],
                                start=(dc == 0), stop=(dc == DC - 1),
                            )
                        nc.vector.tensor_scalar_add(
                            out=dst[:, 512 * tcn:512 * (tcn + 1)],
                            in0=ps[:],
                            scalar1=b_sb[:, j:j + 1],
                        )

            def emit_v_proj(j, tt_lo, tt_hi):
                """V rows for pair j: psum [t=128, f=128] per t-tile."""
                for tt in range(tt_lo, tt_hi):
                    ps = qkv_pool.tile([P, P], fp32, tag="qkv")
                    for dc in range(DC):
                        nc.tensor.matmul(
                            ps[:],
                            xt_sb[:, dc, P * tt:P * (tt + 1)],
                            wv_sb[:, dc, P * j:P * (j + 1)],
                            start=(dc == 0), stop=(dc == DC - 1),
                        )
                    nc.vector.tensor_add(
                        out=v_sb[j][:, tt, :, 0:HD],
                        in0=ps[:].rearrange("p (h d) -> p h d", h=2),
                        in1=bv_sb[:, P * j:P * (j + 1)].rearrange(
                            "p (h d) -> p h d", h=2),
                    )

            emit_qk_proj(0)
            emit_v_proj(0, 0, NKT)

            for j in range(NPAIR):
                qt, kt, vv = qt_sb[j], kt_sb[j], v_sb[j]
                for qc in range(NQC):
                    q0 = QC * qc
                    pva = pv_pool.tile([HD + 1, QC], fp32, tag="pva")
                    pvb = pv_pool.tile([HD + 1, QC], fp32, tag="pvb")
                    if qc == 0:
                        sta = stage_pool.tile([HD, T], fp32, tag="sta")
                        stb = stage_pool.tile([HD, T], fp32, tag="stb")
                    for g in range(NKT // 2):
                        kt0, kt1 = 2 * g, 2 * g + 1
                        sA = sps_pool.tile([P, 2, QC], fp32, tag="sps")
                        sB = sps_pool.tile([P, 2, QC], fp32, tag="sps")
                        # scores S^T[k, q]; A on PE rows 0-63, B on 64-127,
                        # interleaved so the row-disjoint matmuls overlap
                        for i, ktn in enumerate((kt0, kt1)):
                            for hp, s in ((0, sA), (1, sB)):
                                nc.tensor.matmul(
                                    s[:, i, :],
                                    kt[HD * hp:HD * (hp + 1),
                                       P * ktn:P * (ktn + 1)],
                                    qt[HD * hp:HD * (hp + 1), q0:q0 + QC],
                                    start=True, stop=True,
                                )
                        esA = es_pool.tile([P, 2, QC], bf16, tag="es")
                        esB = es_pool.tile([P, 2, QC], bf16, tag="es")
                        nc.scalar.activation(
                            esA[:].rearrange("p a b -> p (a b)"),
                            sA[:].rearrange("p a b -> p (a b)"),
                            Exp, scale=0.125)
                        nc.scalar.activation(
                            esB[:].rearrange("p a b -> p (a b)"),
                            sB[:].rearrange("p a b -> p (a b)"),
                            Exp, scale=0.125)
                        for i, ktn in enumerate((kt0, kt1)):
                            first = ktn == 0
                            last = ktn == NKT - 1
                            nc.tensor.matmul(
                                pva[:], vv[:, ktn, 0, :], esA[:, i, :],
                                start=first, stop=last)
                            nc.tensor.matmul(
                                pvb[:], vv[:, ktn, 1, :], esB[:, i, :],
                                start=first, stop=last)
                    # normalize: row HD of pv holds Z = sum_k exp(s/8)
                    for hp, pv_t, st in ((0, pva, sta), (1, pvb, stb)):
                        # Z sits on partition 64; partition_broadcast only
                        # reads partition 0 on HW, so bounce it via sb->sb DMA
                        nc.vector.reciprocal(rzs[hp][HD:HD + 1, :],
                                             pv_t[HD:HD + 1, :])
                        nc.sync.dma_start(out=rz0[hp][:],
                                          in_=rzs[hp][HD:HD + 1, :])
                        nc.gpsimd.partition_broadcast(rzb[hp][:], rz0[hp][:])
                        nc.vector.tensor_mul(st[:, q0:q0 + QC],
                                             pv_t[0:HD, :], rzb[hp][:])
                    # feed the PE pipeline with next pair's projections
                    if j + 1 < NPAIR:
                        if NQC >= 4:
                            if qc == 0:
                                emit_qk_proj(j + 1)
                            elif qc == 1:
                                emit_v_proj(j + 1, 0, NKT // 2)
                            elif qc == 2:
                                emit_v_proj(j + 1, NKT // 2, NKT)
                        elif qc == 0:
                            emit_qk_proj(j + 1)
                            emit_v_proj(j + 1, 0, NKT)
                    if qc == NQC - 1:
                        nc.sync.dma_start(out=o[2 * j], in_=sta[:])
                        nc.sync.dma_start(out=o[2 * j + 1], in_=stb[:])

    nc.compile()
    return nc


def _prep_inputs(x, Wq, bq, Wk, bk, Wv, bv):
    """Host-side shard + layout prep. Returns per-core input dicts."""
    in_maps = []
    xt_cache = {}
    w_cache = {}
    for c in range(N_CORES):
        b, g = c // G, c % G
        if b not in xt_cache:
            xtb = np.ascontiguousarray(x[b].T).astype(BF16)      # [D, T]
            xt_cache[b] = np.ascontiguousarray(
                xtb.reshape(DC, P, T).transpose(1, 0, 2))        # [P, DC, T]
        if g not in w_cache:
            def _w(W):
                Wg = W[:, F * g:F * (g + 1)].astype(BF16)        # [D, F]
                return np.ascontiguousarray(
                    Wg.reshape(DC, P, F).transpose(1, 0, 2))     # [P, DC, F]
            bqg = bq[F * g:F * (g + 1)].astype(np.float32)
            bkg = bk[F * g:F * (g + 1)].astype(np.float32)
            bvg = bv[F * g:F * (g + 1)].astype(np.float32)
            w_cache[g] = {
                "wq": _w(Wq), "wk": _w(Wk), "wv": _w(Wv),
                # [P, NPAIR]: bias for feature 128*j + p
                "bq": np.ascontiguousarray(bqg.reshape(NPAIR, P).T),
                "bk": np.ascontiguousarray(bkg.reshape(NPAIR, P).T),
                # [P, F]: broadcast along partitions
                "bv": np.ascontiguousarray(
                    np.broadcast_to(bvg[None, :], (P, F))),
            }
        in_maps.append({"xt": xt_cache[b], **w_cache[g]})
    return in_maps


def _run(in_maps, trace_dir=None, trace_cores=None):
    from concourse.bass_utils import run_bass_kernel_spmd

    global _compiled
    if _compiled is None:
        _compiled = _build()
    nc = _compiled

    if trace_dir is not None:
        from trn_agent_boot.trn_boot import _ntff_profile_via_ctypes
        hook = _ntff_profile_via_ctypes("/opt/axon/libaxon_pjrt.so")
        with hook(trace_dir, trace_cores):
            res = run_bass_kernel_spmd(nc, in_maps,
                                       core_ids=list(range(N_CORES)))
    else:
        res = run_bass_kernel_spmd(nc, in_maps, core_ids=list(range(N_CORES)))
    return res


def kernel(x, Wq, bq, Wk, bk, Wv, bv, _trace_dir=None, _trace_cores=None):
    x = np.asarray(x, dtype=np.float32)
    in_maps = _prep_inputs(x, np.asarray(Wq), np.asarray(bq), np.asarray(Wk),
                           np.asarray(bk), np.asarray(Wv), np.asarray(bv))
    res = _run(in_maps, _trace_dir, _trace_cores)
    out = np.empty((B, T, D), np.float32)
    for c in range(N_CORES):
        b, g = c // G, c % G
        oc = np.asarray(res.results[c]["o"])          # [HPC, HD, T]
        out[b, :, F * g:F * (g + 1)] = (
            oc.transpose(2, 0, 1).reshape(T, F))
    return out
